# revision 18
# baseline (speedup 1.0000x reference)
"""Trainium2 Bass kernel for nn_CONV_tiny_add_partial_558345748883.

Network: 3x [conv5x5(pad2) -> BN -> avgpool2 -> clip01] -> conv4x4(valid) -> BN1d
Input x_in [1024, 3, 32, 32] f32; output [1024, 10] f32.

Strategy
--------
- Data parallel: batch 1024 split over 8 NeuronCores (128 samples each).
- Each conv+BN+pool block is algebraically folded into one stride-2 6x6 conv
  (pooling/BN are linear: pool(bn(conv(x))) == stride2conv(x; W') + beta),
  cutting PE work ~2.8x and removing all pooling vector work.
- BN scale folds into conv weights; BN bias + lower clip via ScalarE
  Relu(x + beta) on PSUM eviction; upper clip via one VectorE
  tensor_scalar_min over each layer tile.
- PE mapping: "diagonal lanes". Partition groups j=0..3 each own one sample
  stream (sample s -> lane s%4). Convs run as 32x32 (L1/L2) / 32x64 (L3)
  tensor-engine sub-tiles at explicit tile_position, one matmul per kernel
  tap, all taps of a sample accumulating into one PSUM bank. The four lanes
  execute concurrently on disjoint PE sub-arrays.
- L1 contraction packed to K=18 = (6 dy) x (3 ci): dy shifts + stride-2 row
  decimation baked into a host-side im2row layout, so L1 runs just 6 dx taps.
- float16 matmul operands (full PE rate, fp32 PSUM accumulation).
"""
import os
import sys
import numpy as np

for _p in ("/opt/trn_rl_repo", "/root/.axon_site/_ro/trn_rl_repo"):
    if os.path.isdir(_p) and _p not in sys.path:
        sys.path.append(_p)

import concourse.bass as bass
import concourse.bacc as bacc
import concourse.mybir as mybir
from concourse.tile import TileContext

EPS = 1e-5
N_CORES = 8
DT = mybir.dt.float16
F32 = mybir.dt.float32
AF = mybir.ActivationFunctionType

# sizes (mutable via configure() for small-scale simulation tests)
NW = 2    # waves per core
Q = 16    # samples per lane per wave
S = NW * 4 * Q          # samples per core
HQ = Q // 2             # samples per lane per half-wave (L1 dma granularity)
NQ = NW * Q             # per-lane total samples
USE_CLEARS = False  # True: belt-and-braces PSUM bank clear matmuls (needed for CoreSim's
                    # bank-granular accumulation-group model; HW has_written is per-partition)


def configure(nw, q, use_clears=None):
    global NW, Q, S, HQ, NQ, _NC_CACHE, USE_CLEARS
    if use_clears is not None:
        USE_CLEARS = use_clears
    NW, Q = nw, q
    S = NW * 4 * Q
    HQ = Q // 2
    NQ = NW * Q
    _NC_CACHE = None


# ----------------------------------------------------------------------------
# Host-side prep
# ----------------------------------------------------------------------------

def _fold_w(w, g, b, m, v):
    """Fold conv5x5 + BN + avgpool2 into stride-2 6x6 weights + bias."""
    inv = g / np.sqrt(v + EPS)
    Wp = np.zeros((w.shape[0], w.shape[1], 6, 6), np.float32)
    for r in (0, 1):
        for s_ in (0, 1):
            Wp[:, :, r:r + 5, s_:s_ + 5] += w
    Wp *= 0.25 * inv[:, None, None, None]
    beta = (b - m * inv).astype(np.float32)
    return Wp.astype(np.float32), beta


def _lane_rep(a, groups=4):
    """Replicate [p, f] into [128, f] across partition groups of 32."""
    out = np.zeros((128, a.shape[1]), np.float32)
    for j in range(groups):
        out[32 * j:32 * j + a.shape[0]] = a
    return out


def host_prep_weights(inputs):
    W1, beta1 = _fold_w(inputs['w1'], inputs['g1'], inputs['b1'], inputs['m1'], inputs['v1'])
    W2, beta2 = _fold_w(inputs['w2'], inputs['g2'], inputs['b2'], inputs['m2'], inputs['v2'])
    W3, beta3 = _fold_w(inputs['w3'], inputs['g3'], inputs['b3'], inputs['m3'], inputs['v3'])
    inv4 = inputs['g4'] / np.sqrt(inputs['v4'] + EPS)
    beta4 = (inputs['b4'] - inputs['m4'] * inv4).astype(np.float32)
    W4 = (inputs['w4'] * inv4[:, None, None, None]).astype(np.float32)

    d = {}
    # L1 lhsT per dx tap f: wl1[dy*3+ci, f*32+co] = W1[co, ci, dy, f]
    d['wl1'] = _lane_rep(W1.transpose(2, 1, 3, 0).reshape(18, 6 * 32)).astype(np.float16)
    # L2 lhsT per tap t=e*6+f: [32 ci, 32 co]
    d['wl2'] = _lane_rep(W2.transpose(1, 2, 3, 0).reshape(32, 36 * 32)).astype(np.float16)
    # L3 lhsT per tap: [32 ci, 64 co]
    d['wl3'] = _lane_rep(W3.transpose(1, 2, 3, 0).reshape(32, 36 * 64)).astype(np.float16)
    # L4 lhsT per tap t=u*4+v: [64 ci, 10 co], replicated into both row halves
    wl4 = W4.transpose(1, 2, 3, 0).reshape(64, 16 * 10)
    out4 = np.zeros((128, 160), np.float32)
    out4[0:64] = wl4
    out4[64:128] = wl4
    d['wl4'] = out4.astype(np.float16)

    bt = np.zeros((128, 4), np.float32)
    bt[:, 0] = np.tile(beta1, 4)
    bt[:, 1] = np.tile(beta2, 4)
    bt[:, 2] = np.tile(beta3, 2)
    bt[0:10, 3] = beta4
    d['betas'] = bt
    return d


def host_prep_x(x_core):
    """[S, 3, 32, 32] -> x_l1 [4, 18, NQ, 576] im2row layout.

    x_l1[lane, dy*3+ci, qg, r*36+c] = xpad[4*qg+lane, ci, 2r+dy, c]
    """
    Sc = x_core.shape[0]
    xp = np.zeros((Sc, 3, 36, 36), np.float32)
    xp[:, :, 2:34, 2:34] = x_core
    arr = np.stack([xp[:, :, dy:dy + 32:2, :] for dy in range(6)], axis=1)  # [S,6,3,16,36]
    arr = arr.reshape(Sc, 18, 16 * 36)
    x_l1 = arr.reshape(Sc // 4, 4, 18, 576).transpose(1, 2, 0, 3)
    return np.ascontiguousarray(x_l1).astype(np.float16)


# ----------------------------------------------------------------------------
# Bass program
# ----------------------------------------------------------------------------

def build_program():
    nc = bacc.Bacc(target_bir_lowering=False)

    x_l1 = nc.dram_tensor("x_l1", [4, 18, NQ, 576], DT, kind="ExternalInput")
    wl1 = nc.dram_tensor("wl1", [128, 192], DT, kind="ExternalInput")
    wl2 = nc.dram_tensor("wl2", [128, 1152], DT, kind="ExternalInput")
    wl3 = nc.dram_tensor("wl3", [128, 2304], DT, kind="ExternalInput")
    wl4 = nc.dram_tensor("wl4", [128, 160], DT, kind="ExternalInput")
    betas = nc.dram_tensor("betas", [128, 4], F32, kind="ExternalInput")
    y = nc.dram_tensor("y", [10, 4 * NQ], F32, kind="ExternalOutput")

    TAPS = [(e, f) for e in range(6) for f in range(6)]

    with TileContext(nc) as tc:
        with tc.tile_pool(name="consts", bufs=1) as cpool:
            # ---- constants ----
            wl1_t = cpool.tile([128, 192], DT, name="wl1_t")
            wl2_t = cpool.tile([128, 1152], DT, name="wl2_t")
            wl3_t = cpool.tile([128, 2304], DT, name="wl3_t")
            wl4_t = cpool.tile([128, 160], DT, name="wl4_t")
            betas_t = cpool.tile([128, 4], F32, name="betas_t")
            nc.sync.dma_start(wl1_t[:, :], wl1.ap())
            nc.sync.dma_start(wl2_t[:, :], wl2.ap())
            nc.sync.dma_start(wl3_t[:, :], wl3.ap())
            nc.sync.dma_start(wl4_t[:, :], wl4.ap())
            nc.sync.dma_start(betas_t[:, :], betas.ap())

            # ---- persistent activation tiles (manual double buffer) ----
            l1in = [cpool.tile([128, Q * 576], DT, name=f"l1in{i}") for i in range(2)]
            l2in = [cpool.tile([128, Q * 400], DT, name=f"l2in{i}") for i in range(2)]
            l3in = [cpool.tile([128, Q * 144], DT, name=f"l3in{i}") for i in range(2)]
            stagA = cpool.tile([128, NQ * 16], DT, name="stagA")
            stagB = cpool.tile([128, NQ * 16], DT, name="stagB")
            out_sb = cpool.tile([128, 4 * NQ], F32, name="out_sb")

            # zero padded-window buffers once; interiors are rewritten each
            # wave, borders stay zero. Wave-0 buffers first (they gate the
            # first evacs).
            for t_ in (l2in[0], l3in[0], l2in[1], l3in[1]):
                nc.vector.memset(t_[:, :], 0.0)

            # zeros used by full-width PSUM-clearing matmuls (one accumulation
            # group per bank: a [128, N] start=True matmul claims + clears the
            # bank, then per-lane tap matmuls accumulate with start=False).
            zeros_t = cpool.tile([1, 768], DT, name="zeros_t")
            if USE_CLEARS:
                nc.vector.memset(zeros_t[:, :], 0.0)

            def psum_clear(ps, n):
                if USE_CLEARS:
                    nc.tensor.matmul(
                        ps[0:128, 0:n], zeros_t[0:1, 0:128], zeros_t[0:1, 128:128 + n],
                        start=True, stop=True, tile_position=(0, 0),
                    )

            with (
                tc.tile_pool(name="ps1", bufs=2, space="PSUM") as ps1pool,
                tc.tile_pool(name="ps2", bufs=2, space="PSUM") as ps2pool,
                tc.tile_pool(name="ps3", bufs=1, space="PSUM") as ps3pool,
            ):
                ps3_tiles = [
                    ps3pool.tile([128, Q * 16], F32, name=f"ps3_{j}", tag=f"ps3_{j}")
                    for j in range(4)
                ]

                for w in range(NW):
                    l2t, l3t = l2in[w % 2], l3in[w % 2]
                    l1t = l1in[w % 2]
                    l1d = l1t.rearrange("p (s v) -> p s v", v=576)
                    # ---- L1 input DMA: fine-grained at the very start so the
                    # first matmuls can begin almost immediately, coarse after.
                    if w == 0:
                        for b in range(2):
                            for j in range(4):
                                nc.sync.dma_start(
                                    l1d[32 * j:32 * j + 18, 2 * b:2 * b + 2, :],
                                    x_l1.ap()[j, :, 2 * b:2 * b + 2, :],
                                )
                        if Q > 4:
                            for j in range(4):
                                nc.sync.dma_start(
                                    l1d[32 * j:32 * j + 18, 4:Q, :],
                                    x_l1.ap()[j, :, 4:Q, :],
                                )
                    else:
                        for j in range(4):
                            nc.sync.dma_start(
                                l1d[32 * j:32 * j + 18, :, :],
                                x_l1.ap()[j, :, w * Q:(w + 1) * Q, :],
                            )
                    l1v = l1t.rearrange("p (s r c) -> p s r c", s=Q, r=16)
                    # ================= L1 =================
                    for b in range(Q // 2):  # subwaves: 2 samples/lane
                        ps1 = ps1pool.tile([128, 512], F32, name="ps1", tag="ps1")
                        psum_clear(ps1, 512)
                        for f in range(6):
                            for j in range(4):
                                lhsT = wl1_t[32 * j:32 * j + 18, 32 * f:32 * f + 32]
                                rhs = l1v[32 * j:32 * j + 18, 2 * b:2 * b + 2, :, f:f + 31:2]
                                nc.tensor.matmul(
                                    ps1[32 * j:32 * j + 32, :], lhsT, rhs,
                                    start=(not USE_CLEARS and f == 0), stop=False,
                                    skip_group_check=True,
                                    tile_position=(32 * j, 32 * j),
                                )
                        # evac: Relu(x + beta1) -> l2 window interiors
                        qb = 2 * b
                        src = ps1.rearrange("p (s yy xx) -> p s yy xx", s=2, yy=16)
                        dst = l2t.rearrange("p (s yy xx) -> p s yy xx", s=Q, yy=20)
                        nc.scalar.activation(
                            dst[:, qb:qb + 2, 2:18, 2:18], src,
                            AF.Relu, bias=betas_t[:, 0:1], scale=1.0,
                        )
                    nc.vector.tensor_scalar_min(l2t[:, :], l2t[:, :], 1.0)

                    # ================= L2 =================
                    l2v = l2t.rearrange("p (s yy xx) -> p s yy xx", s=Q, yy=20)
                    G2 = min(8, Q)
                    for a in range(Q // G2):
                        ps2 = ps2pool.tile([128, G2 * 64], F32, name="ps2", tag="ps2")
                        psum_clear(ps2, G2 * 64)
                        for t, (e, f) in enumerate(TAPS):
                            for j in range(4):
                                lhsT = wl2_t[32 * j:32 * j + 32, 32 * t:32 * t + 32]
                                rhs = l2v[32 * j:32 * j + 32, G2 * a:G2 * (a + 1),
                                          e:e + 15:2, f:f + 15:2]
                                nc.tensor.matmul(
                                    ps2[32 * j:32 * j + 32, :], lhsT, rhs,
                                    start=(not USE_CLEARS and t == 0), stop=False,
                                    skip_group_check=True,
                                    tile_position=(32 * j, 32 * j),
                                )
                        src = ps2.rearrange("p (s yy xx) -> p s yy xx", s=G2, yy=8)
                        dst = l3t.rearrange("p (s yy xx) -> p s yy xx", s=Q, yy=12)
                        nc.scalar.activation(
                            dst[:, G2 * a:G2 * (a + 1), 2:10, 2:10], src,
                            AF.Relu, bias=betas_t[:, 1:2], scale=1.0,
                        )
                    nc.vector.tensor_scalar_min(l3t[:, :], l3t[:, :], 1.0)

                    # ================= L3 =================
                    l3v = l3t.rearrange("p (s yy xx) -> p s yy xx", s=Q, yy=12)
                    for t, (e, f) in enumerate(TAPS):
                        for j in range(4):
                            c = j // 2
                            lhsT = wl3_t[32 * j:32 * j + 32, 64 * t:64 * t + 64]
                            rhs = l3v[32 * j:32 * j + 32, :, e:e + 7:2, f:f + 7:2]
                            nc.tensor.matmul(
                                ps3_tiles[j][64 * c:64 * c + 64, :], lhsT, rhs,
                                start=(t == 0), stop=(t == 35),
                                skip_group_check=True,
                                tile_position=(32 * j, 64 * c),
                            )
                    for j in range(4):
                        c = j // 2
                        stag = stagA if j % 2 == 0 else stagB
                        nc.scalar.activation(
                            stag[64 * c:64 * c + 64, w * Q * 16:(w + 1) * Q * 16],
                            ps3_tiles[j][64 * c:64 * c + 64, :],
                            AF.Relu, bias=betas_t[64 * c:64 * c + 64, 2:3], scale=1.0,
                        )

            nc.vector.tensor_scalar_min(stagA[:, :], stagA[:, :], 1.0)
            nc.vector.tensor_scalar_min(stagB[:, :], stagB[:, :], 1.0)

            # ================= L4 =================
            with tc.tile_pool(name="ps4", bufs=1, space="PSUM") as ps4pool:
                streams = [(stagA, 0), (stagA, 1), (stagB, 0), (stagB, 1)]
                ps4s = [ps4pool.tile([128, NQ], F32, name=f"ps4_{k}", tag=f"ps4_{k}")
                        for k in range(4)]
                for t in range(16):
                    for k, (stag, r) in enumerate(streams):
                        sv = stag.rearrange("p (n t) -> p n t", t=16)
                        lhsT = wl4_t[64 * r:64 * r + 64, 10 * t:10 * t + 10]
                        rhs = sv[64 * r:64 * r + 64, :, t]
                        nc.tensor.matmul(
                            ps4s[k][0:10, :], lhsT, rhs,
                            start=(t == 0), stop=(t == 15),
                            skip_group_check=True,
                            tile_position=(64 * r, 0),
                        )
                for k in range(4):
                    nc.scalar.activation(
                        out_sb[0:10, k * NQ:(k + 1) * NQ], ps4s[k][0:10, :],
                        AF.Identity, bias=betas_t[0:10, 3:4], scale=1.0,
                    )
                nc.sync.dma_start(y.ap(), out_sb[0:10, :])

    return nc


_NC_CACHE = None


def get_program():
    global _NC_CACHE
    if _NC_CACHE is None:
        nc = build_program()
        if not nc.is_finalized():
            nc.finalize()
        _NC_CACHE = nc
    return _NC_CACHE


def make_in_maps(inputs, n_cores=N_CORES):
    wdict = host_prep_weights(inputs)
    in_maps = []
    for c in range(n_cores):
        x_core = np.asarray(inputs['x_in'][c * S:(c + 1) * S], np.float32)
        m = {'x_l1': host_prep_x(x_core)}
        m.update(wdict)
        in_maps.append(m)
    return in_maps


def assemble_output(results, n_cores=N_CORES):
    """results: list of per-core dicts with y [10, 4*NQ] -> [n_cores*S, 10]."""
    out = np.zeros((n_cores * S, 10), np.float32)
    lanes = [0, 2, 1, 3]
    for c in range(n_cores):
        yc = np.asarray(results[c]['y'])  # [10, 4*NQ]
        for k, lane in enumerate(lanes):
            blk = yc[:, k * NQ:(k + 1) * NQ]  # [10, NQ]
            s_core = 4 * np.arange(NQ) + lane
            out[c * S + s_core, :] = blk.T
    return out


def kernel(**inputs) -> np.ndarray:
    from concourse.bass_utils import run_bass_kernel_spmd
    nc = get_program()
    in_maps = make_in_maps(inputs)
    res = run_bass_kernel_spmd(nc, in_maps, list(range(N_CORES)))
    return assemble_output(res.results)


# revision 19
# speedup vs baseline: 1.0397x; 1.0397x over previous
"""Trainium2 Bass kernel for nn_CONV_tiny_add_partial_558345748883.

Network: 3x [conv5x5(pad2) -> BN -> avgpool2 -> clip01] -> conv4x4(valid) -> BN1d
Input x_in [1024, 3, 32, 32] f32; output [1024, 10] f32.

Strategy
--------
- Data parallel: batch 1024 split over 8 NeuronCores (128 samples each).
- Each conv+BN+pool block is algebraically folded into one stride-2 6x6 conv
  (pooling/BN are linear: pool(bn(conv(x))) == stride2conv(x; W') + beta),
  cutting PE work ~2.8x and removing all pooling vector work.
- BN scale folds into conv weights; BN bias + lower clip via ScalarE
  Relu(x + beta) on PSUM eviction; upper clip via one VectorE
  tensor_scalar_min over each layer tile.
- PE mapping: "diagonal lanes". Partition groups j=0..3 each own one sample
  stream (sample s -> lane s%4). Convs run as 32x32 (L1/L2) / 32x64 (L3)
  tensor-engine sub-tiles at explicit tile_position, one matmul per kernel
  tap, all taps of a sample accumulating into one PSUM bank. The four lanes
  execute concurrently on disjoint PE sub-arrays.
- L1 contraction packed to K=18 = (6 dy) x (3 ci): dy shifts + stride-2 row
  decimation baked into a host-side im2row layout, so L1 runs just 6 dx taps.
- float16 matmul operands (full PE rate, fp32 PSUM accumulation).
"""
import os
import sys
import numpy as np

for _p in ("/opt/trn_rl_repo", "/root/.axon_site/_ro/trn_rl_repo"):
    if os.path.isdir(_p) and _p not in sys.path:
        sys.path.append(_p)

import concourse.bass as bass
import concourse.bacc as bacc
import concourse.mybir as mybir
from concourse.tile import TileContext

EPS = 1e-5
N_CORES = 8
DT = mybir.dt.float16
F32 = mybir.dt.float32
AF = mybir.ActivationFunctionType

# sizes (mutable via configure() for small-scale simulation tests)
NW = 2    # waves per core
Q = 16    # samples per lane per wave
S = NW * 4 * Q          # samples per core
HQ = Q // 2             # samples per lane per half-wave (L1 dma granularity)
NQ = NW * Q             # per-lane total samples
USE_CLEARS = False  # True: belt-and-braces PSUM bank clear matmuls (needed for CoreSim's
                    # bank-granular accumulation-group model; HW has_written is per-partition)


def configure(nw, q, use_clears=None):
    global NW, Q, S, HQ, NQ, _NC_CACHE, USE_CLEARS
    if use_clears is not None:
        USE_CLEARS = use_clears
    NW, Q = nw, q
    S = NW * 4 * Q
    HQ = Q // 2
    NQ = NW * Q
    _NC_CACHE = None


# ----------------------------------------------------------------------------
# Host-side prep
# ----------------------------------------------------------------------------

def _fold_w(w, g, b, m, v):
    """Fold conv5x5 + BN + avgpool2 into stride-2 6x6 weights + bias."""
    inv = g / np.sqrt(v + EPS)
    Wp = np.zeros((w.shape[0], w.shape[1], 6, 6), np.float32)
    for r in (0, 1):
        for s_ in (0, 1):
            Wp[:, :, r:r + 5, s_:s_ + 5] += w
    Wp *= 0.25 * inv[:, None, None, None]
    beta = (b - m * inv).astype(np.float32)
    return Wp.astype(np.float32), beta


def _lane_rep(a, groups=4):
    """Replicate [p, f] into [128, f] across partition groups of 32."""
    out = np.zeros((128, a.shape[1]), np.float32)
    for j in range(groups):
        out[32 * j:32 * j + a.shape[0]] = a
    return out


def host_prep_weights(inputs):
    W1, beta1 = _fold_w(inputs['w1'], inputs['g1'], inputs['b1'], inputs['m1'], inputs['v1'])
    W2, beta2 = _fold_w(inputs['w2'], inputs['g2'], inputs['b2'], inputs['m2'], inputs['v2'])
    W3, beta3 = _fold_w(inputs['w3'], inputs['g3'], inputs['b3'], inputs['m3'], inputs['v3'])
    inv4 = inputs['g4'] / np.sqrt(inputs['v4'] + EPS)
    beta4 = (inputs['b4'] - inputs['m4'] * inv4).astype(np.float32)
    W4 = (inputs['w4'] * inv4[:, None, None, None]).astype(np.float32)

    d = {}
    # L1 lhsT per dx tap f: wl1[dy*3+ci, f*32+co] = W1[co, ci, dy, f]
    d['wl1'] = _lane_rep(W1.transpose(2, 1, 3, 0).reshape(18, 6 * 32)).astype(np.float16)
    # L2 lhsT per tap t=e*6+f: [32 ci, 32 co]
    d['wl2'] = _lane_rep(W2.transpose(1, 2, 3, 0).reshape(32, 36 * 32)).astype(np.float16)
    # L3 lhsT per tap: [32 ci, 64 co]
    d['wl3'] = _lane_rep(W3.transpose(1, 2, 3, 0).reshape(32, 36 * 64)).astype(np.float16)
    # L4 lhsT per tap t=u*4+v: [64 ci, 10 co], replicated into both row halves
    wl4 = W4.transpose(1, 2, 3, 0).reshape(64, 16 * 10)
    out4 = np.zeros((128, 160), np.float32)
    out4[0:64] = wl4
    out4[64:128] = wl4
    d['wl4'] = out4.astype(np.float16)

    bt = np.zeros((128, 4), np.float32)
    bt[:, 0] = np.tile(beta1, 4)
    bt[:, 1] = np.tile(beta2, 4)
    bt[:, 2] = np.tile(beta3, 2)
    bt[0:10, 3] = beta4
    d['betas'] = bt
    return d


def host_prep_x(x_core):
    """[S, 3, 32, 32] -> x_l1 [4, 18, NQ, 576] im2row layout.

    x_l1[lane, dy*3+ci, qg, r*36+c] = xpad[4*qg+lane, ci, 2r+dy, c]
    """
    Sc = x_core.shape[0]
    xp = np.zeros((Sc, 3, 36, 36), np.float32)
    xp[:, :, 2:34, 2:34] = x_core
    arr = np.stack([xp[:, :, dy:dy + 32:2, :] for dy in range(6)], axis=1)  # [S,6,3,16,36]
    arr = arr.reshape(Sc, 18, 16 * 36)
    x_l1 = arr.reshape(Sc // 4, 4, 18, 576).transpose(1, 2, 0, 3)
    return np.ascontiguousarray(x_l1).astype(np.float16)


# ----------------------------------------------------------------------------
# Bass program
# ----------------------------------------------------------------------------

def build_program():
    nc = bacc.Bacc(target_bir_lowering=False)

    x_l1 = nc.dram_tensor("x_l1", [4, 18, NQ, 576], DT, kind="ExternalInput")
    wl1 = nc.dram_tensor("wl1", [128, 192], DT, kind="ExternalInput")
    wl2 = nc.dram_tensor("wl2", [128, 1152], DT, kind="ExternalInput")
    wl3 = nc.dram_tensor("wl3", [128, 2304], DT, kind="ExternalInput")
    wl4 = nc.dram_tensor("wl4", [128, 160], DT, kind="ExternalInput")
    betas = nc.dram_tensor("betas", [128, 4], F32, kind="ExternalInput")
    y = nc.dram_tensor("y", [10, 4 * NQ], F32, kind="ExternalOutput")

    TAPS = [(e, f) for e in range(6) for f in range(6)]

    with TileContext(nc) as tc:
        with tc.tile_pool(name="consts", bufs=1) as cpool:
            # ---- constants ----
            wl1_t = cpool.tile([128, 192], DT, name="wl1_t")
            wl2_t = cpool.tile([128, 1152], DT, name="wl2_t")
            wl3_t = cpool.tile([128, 2304], DT, name="wl3_t")
            wl4_t = cpool.tile([128, 160], DT, name="wl4_t")
            betas_t = cpool.tile([128, 4], F32, name="betas_t")
            nc.sync.dma_start(wl1_t[:, :], wl1.ap())
            nc.sync.dma_start(wl2_t[:, :], wl2.ap())
            nc.sync.dma_start(wl3_t[:, :], wl3.ap())
            nc.sync.dma_start(wl4_t[:, :], wl4.ap())
            nc.sync.dma_start(betas_t[:, :], betas.ap())

            # ---- persistent activation tiles (manual double buffer) ----
            l1in = [cpool.tile([128, Q * 576], DT, name=f"l1in{i}") for i in range(2)]
            l2in = [cpool.tile([128, Q * 400], DT, name=f"l2in{i}") for i in range(2)]
            l3in = [cpool.tile([128, Q * 144], DT, name=f"l3in{i}") for i in range(2)]
            stagA = cpool.tile([128, NQ * 16], DT, name="stagA")
            stagB = cpool.tile([128, NQ * 16], DT, name="stagB")
            out_sb = cpool.tile([128, 4 * NQ], F32, name="out_sb")

            # zero padded-window buffers once; interiors are rewritten each
            # wave, borders stay zero. Wave-0 buffers first (they gate the
            # first evacs).
            for t_ in (l2in[0], l3in[0], l2in[1], l3in[1]):
                nc.vector.memset(t_[:, :], 0.0)

            # zeros used by full-width PSUM-clearing matmuls (one accumulation
            # group per bank: a [128, N] start=True matmul claims + clears the
            # bank, then per-lane tap matmuls accumulate with start=False).
            zeros_t = cpool.tile([1, 768], DT, name="zeros_t")
            if USE_CLEARS:
                nc.vector.memset(zeros_t[:, :], 0.0)

            def psum_clear(ps, n):
                if USE_CLEARS:
                    nc.tensor.matmul(
                        ps[0:128, 0:n], zeros_t[0:1, 0:128], zeros_t[0:1, 128:128 + n],
                        start=True, stop=True, tile_position=(0, 0),
                    )

            with (
                tc.tile_pool(name="ps1", bufs=2, space="PSUM") as ps1pool,
                tc.tile_pool(name="ps2", bufs=2, space="PSUM") as ps2pool,
                tc.tile_pool(name="ps3", bufs=1, space="PSUM") as ps3pool,
            ):
                ps3_tiles = [
                    ps3pool.tile([128, Q * 16], F32, name=f"ps3_{j}", tag=f"ps3_{j}")
                    for j in range(4)
                ]

                for w in range(NW):
                    l2t, l3t = l2in[w % 2], l3in[w % 2]
                    l1t = l1in[w % 2]
                    l1d = l1t.rearrange("p (s v) -> p s v", v=576)
                    # ---- L1 input DMA: fine-grained at the very start so the
                    # first matmuls can begin almost immediately, coarse after.
                    if w == 0:
                        # fine-grained start: 2-sample chunks for the first
                        # quarter, then quarter-wave chunks
                        chunks = [(0, 2), (2, 4)] + [(c, c + 4) for c in range(4, Q, 4)]
                    else:
                        chunks = [(c, c + 4) for c in range(0, Q, 4)]
                    for c0, c1 in chunks:
                        for j in range(4):
                            nc.sync.dma_start(
                                l1d[32 * j:32 * j + 18, c0:c1, :],
                                x_l1.ap()[j, :, w * Q + c0:w * Q + c1, :],
                            )
                    l1v = l1t.rearrange("p (s r c) -> p s r c", s=Q, r=16)
                    # ================= L1 =================
                    for b in range(Q // 2):  # subwaves: 2 samples/lane
                        ps1 = ps1pool.tile([128, 512], F32, name="ps1", tag="ps1")
                        psum_clear(ps1, 512)
                        for f in range(6):
                            for j in range(4):
                                lhsT = wl1_t[32 * j:32 * j + 18, 32 * f:32 * f + 32]
                                rhs = l1v[32 * j:32 * j + 18, 2 * b:2 * b + 2, :, f:f + 31:2]
                                nc.tensor.matmul(
                                    ps1[32 * j:32 * j + 32, :], lhsT, rhs,
                                    start=(not USE_CLEARS and f == 0), stop=False,
                                    skip_group_check=True,
                                    tile_position=(32 * j, 32 * j),
                                )
                        # evac: Relu(x + beta1) -> l2 window interiors
                        qb = 2 * b
                        src = ps1.rearrange("p (s yy xx) -> p s yy xx", s=2, yy=16)
                        dst = l2t.rearrange("p (s yy xx) -> p s yy xx", s=Q, yy=20)
                        nc.scalar.activation(
                            dst[:, qb:qb + 2, 2:18, 2:18], src,
                            AF.Relu, bias=betas_t[:, 0:1], scale=1.0,
                        )
                    nc.vector.tensor_scalar_min(l2t[:, :], l2t[:, :], 1.0)

                    # ================= L2 =================
                    l2v = l2t.rearrange("p (s yy xx) -> p s yy xx", s=Q, yy=20)
                    G2 = min(8, Q)
                    for a in range(Q // G2):
                        ps2 = ps2pool.tile([128, G2 * 64], F32, name="ps2", tag="ps2")
                        psum_clear(ps2, G2 * 64)
                        for t, (e, f) in enumerate(TAPS):
                            for j in range(4):
                                lhsT = wl2_t[32 * j:32 * j + 32, 32 * t:32 * t + 32]
                                rhs = l2v[32 * j:32 * j + 32, G2 * a:G2 * (a + 1),
                                          e:e + 15:2, f:f + 15:2]
                                nc.tensor.matmul(
                                    ps2[32 * j:32 * j + 32, :], lhsT, rhs,
                                    start=(not USE_CLEARS and t == 0), stop=False,
                                    skip_group_check=True,
                                    tile_position=(32 * j, 32 * j),
                                )
                        src = ps2.rearrange("p (s yy xx) -> p s yy xx", s=G2, yy=8)
                        dst = l3t.rearrange("p (s yy xx) -> p s yy xx", s=Q, yy=12)
                        nc.scalar.activation(
                            dst[:, G2 * a:G2 * (a + 1), 2:10, 2:10], src,
                            AF.Relu, bias=betas_t[:, 1:2], scale=1.0,
                        )
                    nc.vector.tensor_scalar_min(l3t[:, :], l3t[:, :], 1.0)

                    # ================= L3 =================
                    l3v = l3t.rearrange("p (s yy xx) -> p s yy xx", s=Q, yy=12)
                    for t, (e, f) in enumerate(TAPS):
                        for j in range(4):
                            c = j // 2
                            lhsT = wl3_t[32 * j:32 * j + 32, 64 * t:64 * t + 64]
                            rhs = l3v[32 * j:32 * j + 32, :, e:e + 7:2, f:f + 7:2]
                            nc.tensor.matmul(
                                ps3_tiles[j][64 * c:64 * c + 64, :], lhsT, rhs,
                                start=(t == 0), stop=(t == 35),
                                skip_group_check=True,
                                tile_position=(32 * j, 64 * c),
                            )
                    for j in range(4):
                        c = j // 2
                        stag = stagA if j % 2 == 0 else stagB
                        nc.scalar.activation(
                            stag[64 * c:64 * c + 64, w * Q * 16:(w + 1) * Q * 16],
                            ps3_tiles[j][64 * c:64 * c + 64, :],
                            AF.Relu, bias=betas_t[64 * c:64 * c + 64, 2:3], scale=1.0,
                        )

            nc.vector.tensor_scalar_min(stagA[:, :], stagA[:, :], 1.0)
            nc.vector.tensor_scalar_min(stagB[:, :], stagB[:, :], 1.0)

            # ================= L4 =================
            with tc.tile_pool(name="ps4", bufs=1, space="PSUM") as ps4pool:
                streams = [(stagA, 0), (stagA, 1), (stagB, 0), (stagB, 1)]
                ps4s = [ps4pool.tile([128, NQ], F32, name=f"ps4_{k}", tag=f"ps4_{k}")
                        for k in range(4)]
                for t in range(16):
                    for k, (stag, r) in enumerate(streams):
                        sv = stag.rearrange("p (n t) -> p n t", t=16)
                        lhsT = wl4_t[64 * r:64 * r + 64, 10 * t:10 * t + 10]
                        rhs = sv[64 * r:64 * r + 64, :, t]
                        nc.tensor.matmul(
                            ps4s[k][0:10, :], lhsT, rhs,
                            start=(t == 0), stop=(t == 15),
                            skip_group_check=True,
                            tile_position=(64 * r, 0),
                        )
                for k in range(4):
                    nc.scalar.activation(
                        out_sb[0:10, k * NQ:(k + 1) * NQ], ps4s[k][0:10, :],
                        AF.Identity, bias=betas_t[0:10, 3:4], scale=1.0,
                    )
                nc.sync.dma_start(y.ap(), out_sb[0:10, :])

    return nc


_NC_CACHE = None


def get_program():
    global _NC_CACHE
    if _NC_CACHE is None:
        nc = build_program()
        if not nc.is_finalized():
            nc.finalize()
        _NC_CACHE = nc
    return _NC_CACHE


def make_in_maps(inputs, n_cores=N_CORES):
    wdict = host_prep_weights(inputs)
    in_maps = []
    for c in range(n_cores):
        x_core = np.asarray(inputs['x_in'][c * S:(c + 1) * S], np.float32)
        m = {'x_l1': host_prep_x(x_core)}
        m.update(wdict)
        in_maps.append(m)
    return in_maps


def assemble_output(results, n_cores=N_CORES):
    """results: list of per-core dicts with y [10, 4*NQ] -> [n_cores*S, 10]."""
    out = np.zeros((n_cores * S, 10), np.float32)
    lanes = [0, 2, 1, 3]
    for c in range(n_cores):
        yc = np.asarray(results[c]['y'])  # [10, 4*NQ]
        for k, lane in enumerate(lanes):
            blk = yc[:, k * NQ:(k + 1) * NQ]  # [10, NQ]
            s_core = 4 * np.arange(NQ) + lane
            out[c * S + s_core, :] = blk.T
    return out


def kernel(**inputs) -> np.ndarray:
    from concourse.bass_utils import run_bass_kernel_spmd
    nc = get_program()
    in_maps = make_in_maps(inputs)
    res = run_bass_kernel_spmd(nc, in_maps, list(range(N_CORES)))
    return assemble_output(res.results)


# revision 20
# speedup vs baseline: 1.0428x; 1.0029x over previous
"""Trainium2 Bass kernel for nn_CONV_tiny_add_partial_558345748883.

Network: 3x [conv5x5(pad2) -> BN -> avgpool2 -> clip01] -> conv4x4(valid) -> BN1d
Input x_in [1024, 3, 32, 32] f32; output [1024, 10] f32.

Strategy
--------
- Data parallel: batch 1024 split over 8 NeuronCores (128 samples each).
- Each conv+BN+pool block is algebraically folded into one stride-2 6x6 conv
  (pooling/BN are linear: pool(bn(conv(x))) == stride2conv(x; W') + beta),
  cutting PE work ~2.8x and removing all pooling vector work.
- BN scale folds into conv weights; BN bias + lower clip via ScalarE
  Relu(x + beta) on PSUM eviction; upper clip via one VectorE
  tensor_scalar_min over each layer tile.
- PE mapping: "diagonal lanes". Partition groups j=0..3 each own one sample
  stream (sample s -> lane s%4). Convs run as 32x32 (L1/L2) / 32x64 (L3)
  tensor-engine sub-tiles at explicit tile_position, one matmul per kernel
  tap, all taps of a sample accumulating into one PSUM bank. The four lanes
  execute concurrently on disjoint PE sub-arrays.
- L1 contraction packed to K=18 = (6 dy) x (3 ci): dy shifts + stride-2 row
  decimation baked into a host-side im2row layout, so L1 runs just 6 dx taps.
- float16 matmul operands (full PE rate, fp32 PSUM accumulation).
"""
import os
import sys
import numpy as np

for _p in ("/opt/trn_rl_repo", "/root/.axon_site/_ro/trn_rl_repo"):
    if os.path.isdir(_p) and _p not in sys.path:
        sys.path.append(_p)

import concourse.bass as bass
import concourse.bacc as bacc
import concourse.mybir as mybir
from concourse.tile import TileContext

EPS = 1e-5
N_CORES = 8
DT = mybir.dt.float16
F32 = mybir.dt.float32
AF = mybir.ActivationFunctionType

# sizes (mutable via configure() for small-scale simulation tests)
NW = 2    # waves per core
Q = 16    # samples per lane per wave
S = NW * 4 * Q          # samples per core
HQ = Q // 2             # samples per lane per half-wave (L1 dma granularity)
NQ = NW * Q             # per-lane total samples
USE_CLEARS = False  # True: belt-and-braces PSUM bank clear matmuls (needed for CoreSim's
                    # bank-granular accumulation-group model; HW has_written is per-partition)


def configure(nw, q, use_clears=None):
    global NW, Q, S, HQ, NQ, _NC_CACHE, USE_CLEARS
    if use_clears is not None:
        USE_CLEARS = use_clears
    NW, Q = nw, q
    S = NW * 4 * Q
    HQ = Q // 2
    NQ = NW * Q
    _NC_CACHE = None


# ----------------------------------------------------------------------------
# Host-side prep
# ----------------------------------------------------------------------------

def _fold_w(w, g, b, m, v):
    """Fold conv5x5 + BN + avgpool2 into stride-2 6x6 weights + bias."""
    inv = g / np.sqrt(v + EPS)
    Wp = np.zeros((w.shape[0], w.shape[1], 6, 6), np.float32)
    for r in (0, 1):
        for s_ in (0, 1):
            Wp[:, :, r:r + 5, s_:s_ + 5] += w
    Wp *= 0.25 * inv[:, None, None, None]
    beta = (b - m * inv).astype(np.float32)
    return Wp.astype(np.float32), beta


def _lane_rep(a, groups=4):
    """Replicate [p, f] into [128, f] across partition groups of 32."""
    out = np.zeros((128, a.shape[1]), np.float32)
    for j in range(groups):
        out[32 * j:32 * j + a.shape[0]] = a
    return out


def host_prep_weights(inputs):
    W1, beta1 = _fold_w(inputs['w1'], inputs['g1'], inputs['b1'], inputs['m1'], inputs['v1'])
    W2, beta2 = _fold_w(inputs['w2'], inputs['g2'], inputs['b2'], inputs['m2'], inputs['v2'])
    W3, beta3 = _fold_w(inputs['w3'], inputs['g3'], inputs['b3'], inputs['m3'], inputs['v3'])
    inv4 = inputs['g4'] / np.sqrt(inputs['v4'] + EPS)
    beta4 = (inputs['b4'] - inputs['m4'] * inv4).astype(np.float32)
    W4 = (inputs['w4'] * inv4[:, None, None, None]).astype(np.float32)

    d = {}
    # L1 lhsT per dx tap f: wl1[dy*3+ci, f*32+co] = W1[co, ci, dy, f]
    d['wl1'] = _lane_rep(W1.transpose(2, 1, 3, 0).reshape(18, 6 * 32)).astype(np.float16)
    # L2 lhsT per tap t=e*6+f: [32 ci, 32 co]
    d['wl2'] = _lane_rep(W2.transpose(1, 2, 3, 0).reshape(32, 36 * 32)).astype(np.float16)
    # L3 lhsT per tap: [32 ci, 64 co]
    d['wl3'] = _lane_rep(W3.transpose(1, 2, 3, 0).reshape(32, 36 * 64)).astype(np.float16)
    # L4 lhsT per tap t=u*4+v: [64 ci, 10 co], replicated into both row halves
    wl4 = W4.transpose(1, 2, 3, 0).reshape(64, 16 * 10)
    out4 = np.zeros((128, 160), np.float32)
    out4[0:64] = wl4
    out4[64:128] = wl4
    d['wl4'] = out4.astype(np.float16)

    bt = np.zeros((128, 4), np.float32)
    bt[:, 0] = np.tile(beta1, 4)
    bt[:, 1] = np.tile(beta2, 4)
    bt[:, 2] = np.tile(beta3, 2)
    bt[0:10, 3] = beta4
    d['betas'] = bt
    return d


def host_prep_x(x_core):
    """[S, 3, 32, 32] -> x_l1 [4, 18, NQ, 576] im2row layout.

    x_l1[lane, dy*3+ci, qg, r*36+c] = xpad[4*qg+lane, ci, 2r+dy, c]
    """
    Sc = x_core.shape[0]
    xp = np.zeros((Sc, 3, 36, 36), np.float32)
    xp[:, :, 2:34, 2:34] = x_core
    arr = np.stack([xp[:, :, dy:dy + 32:2, :] for dy in range(6)], axis=1)  # [S,6,3,16,36]
    arr = arr.reshape(Sc, 18, 16 * 36)
    x_l1 = arr.reshape(Sc // 4, 4, 18, 576).transpose(1, 2, 0, 3)
    return np.ascontiguousarray(x_l1).astype(np.float16)


# ----------------------------------------------------------------------------
# Bass program
# ----------------------------------------------------------------------------

def build_program():
    nc = bacc.Bacc(target_bir_lowering=False)

    x_l1 = nc.dram_tensor("x_l1", [4, 18, NQ, 576], DT, kind="ExternalInput")
    wl1 = nc.dram_tensor("wl1", [128, 192], DT, kind="ExternalInput")
    wl2 = nc.dram_tensor("wl2", [128, 1152], DT, kind="ExternalInput")
    wl3 = nc.dram_tensor("wl3", [128, 2304], DT, kind="ExternalInput")
    wl4 = nc.dram_tensor("wl4", [128, 160], DT, kind="ExternalInput")
    betas = nc.dram_tensor("betas", [128, 4], F32, kind="ExternalInput")
    y = nc.dram_tensor("y", [10, 4 * NQ], F32, kind="ExternalOutput")

    TAPS = [(e, f) for e in range(6) for f in range(6)]

    with TileContext(nc) as tc:
        with tc.tile_pool(name="consts", bufs=1) as cpool:
            # ---- constants ----
            wl1_t = cpool.tile([128, 192], DT, name="wl1_t")
            wl2_t = cpool.tile([128, 1152], DT, name="wl2_t")
            wl3_t = cpool.tile([128, 2304], DT, name="wl3_t")
            wl4_t = cpool.tile([128, 160], DT, name="wl4_t")
            betas_t = cpool.tile([128, 4], F32, name="betas_t")
            nc.sync.dma_start(wl1_t[:, :], wl1.ap())
            nc.sync.dma_start(wl2_t[:, :], wl2.ap())
            nc.sync.dma_start(wl3_t[:, :], wl3.ap())
            nc.sync.dma_start(wl4_t[:, :], wl4.ap())
            nc.sync.dma_start(betas_t[:, :], betas.ap())

            # ---- persistent activation tiles (manual double buffer) ----
            l2in = [cpool.tile([128, Q * 400], DT, name=f"l2in{i}") for i in range(2)]
            l3in = [cpool.tile([128, Q * 144], DT, name=f"l3in{i}") for i in range(2)]
            stagA = cpool.tile([128, NQ * 16], DT, name="stagA")
            stagB = cpool.tile([128, NQ * 16], DT, name="stagB")
            out_sb = cpool.tile([128, 4 * NQ], F32, name="out_sb")

            # zero padded-window buffers once; interiors are rewritten each
            # wave, borders stay zero. Wave-0 buffers first (they gate the
            # first evacs).
            for t_ in (l2in[0], l3in[0], l2in[1], l3in[1]):
                nc.vector.memset(t_[:, :], 0.0)

            # zeros used by full-width PSUM-clearing matmuls (one accumulation
            # group per bank: a [128, N] start=True matmul claims + clears the
            # bank, then per-lane tap matmuls accumulate with start=False).
            zeros_t = cpool.tile([1, 768], DT, name="zeros_t")
            if USE_CLEARS:
                nc.vector.memset(zeros_t[:, :], 0.0)

            def psum_clear(ps, n):
                if USE_CLEARS:
                    nc.tensor.matmul(
                        ps[0:128, 0:n], zeros_t[0:1, 0:128], zeros_t[0:1, 128:128 + n],
                        start=True, stop=True, tile_position=(0, 0),
                    )

            with (
                tc.tile_pool(name="l1io", bufs=4) as l1pool,
                tc.tile_pool(name="ps1", bufs=2, space="PSUM") as ps1pool,
                tc.tile_pool(name="ps2", bufs=2, space="PSUM") as ps2pool,
                tc.tile_pool(name="ps3", bufs=1, space="PSUM") as ps3pool,
            ):
                ps3_tiles = [
                    ps3pool.tile([128, Q * 16], F32, name=f"ps3_{j}", tag=f"ps3_{j}")
                    for j in range(4)
                ]

                GL = min(4, Q)  # samples per lane per L1 input tile
                for w in range(NW):
                    l2t, l3t = l2in[w % 2], l3in[w % 2]
                    # ================= L1 =================
                    for g in range(Q // GL):  # quarter-wave input tiles
                        l1t = l1pool.tile([128, GL * 576], DT, name="l1t", tag="l1t")
                        l1d = l1t.rearrange("p (s v) -> p s v", v=576)
                        q0 = w * Q + g * GL
                        for j in range(4):
                            nc.sync.dma_start(
                                l1d[32 * j:32 * j + 18, :, :],
                                x_l1.ap()[j, :, q0:q0 + GL, :],
                            )
                        l1v = l1t.rearrange("p (s r c) -> p s r c", s=GL, r=16)
                        for b in range(GL // 2):  # subwaves: 2 samples/lane
                            ps1 = ps1pool.tile([128, 512], F32, name="ps1", tag="ps1")
                            psum_clear(ps1, 512)
                            for f in range(6):
                                for j in range(4):
                                    lhsT = wl1_t[32 * j:32 * j + 18, 32 * f:32 * f + 32]
                                    rhs = l1v[32 * j:32 * j + 18, 2 * b:2 * b + 2, :, f:f + 31:2]
                                    nc.tensor.matmul(
                                        ps1[32 * j:32 * j + 32, :], lhsT, rhs,
                                        start=(not USE_CLEARS and f == 0), stop=False,
                                        skip_group_check=True,
                                        tile_position=(32 * j, 32 * j),
                                    )
                            # evac: Relu(x + beta1) -> l2 window interiors
                            qb = g * GL + 2 * b
                            src = ps1.rearrange("p (s yy xx) -> p s yy xx", s=2, yy=16)
                            dst = l2t.rearrange("p (s yy xx) -> p s yy xx", s=Q, yy=20)
                            nc.scalar.activation(
                                dst[:, qb:qb + 2, 2:18, 2:18], src,
                                AF.Relu, bias=betas_t[:, 0:1], scale=1.0,
                            )
                    nc.vector.tensor_scalar_min(l2t[:, :], l2t[:, :], 1.0)

                    # ================= L2 =================
                    l2v = l2t.rearrange("p (s yy xx) -> p s yy xx", s=Q, yy=20)
                    G2 = min(8, Q)
                    for a in range(Q // G2):
                        ps2 = ps2pool.tile([128, G2 * 64], F32, name="ps2", tag="ps2")
                        psum_clear(ps2, G2 * 64)
                        for t, (e, f) in enumerate(TAPS):
                            for j in range(4):
                                lhsT = wl2_t[32 * j:32 * j + 32, 32 * t:32 * t + 32]
                                rhs = l2v[32 * j:32 * j + 32, G2 * a:G2 * (a + 1),
                                          e:e + 15:2, f:f + 15:2]
                                nc.tensor.matmul(
                                    ps2[32 * j:32 * j + 32, :], lhsT, rhs,
                                    start=(not USE_CLEARS and t == 0), stop=False,
                                    skip_group_check=True,
                                    tile_position=(32 * j, 32 * j),
                                )
                        src = ps2.rearrange("p (s yy xx) -> p s yy xx", s=G2, yy=8)
                        dst = l3t.rearrange("p (s yy xx) -> p s yy xx", s=Q, yy=12)
                        nc.scalar.activation(
                            dst[:, G2 * a:G2 * (a + 1), 2:10, 2:10], src,
                            AF.Relu, bias=betas_t[:, 1:2], scale=1.0,
                        )
                    nc.vector.tensor_scalar_min(l3t[:, :], l3t[:, :], 1.0)

                    # ================= L3 =================
                    l3v = l3t.rearrange("p (s yy xx) -> p s yy xx", s=Q, yy=12)
                    for t, (e, f) in enumerate(TAPS):
                        for j in range(4):
                            c = j // 2
                            lhsT = wl3_t[32 * j:32 * j + 32, 64 * t:64 * t + 64]
                            rhs = l3v[32 * j:32 * j + 32, :, e:e + 7:2, f:f + 7:2]
                            nc.tensor.matmul(
                                ps3_tiles[j][64 * c:64 * c + 64, :], lhsT, rhs,
                                start=(t == 0), stop=(t == 35),
                                skip_group_check=True,
                                tile_position=(32 * j, 64 * c),
                            )
                    for j in range(4):
                        c = j // 2
                        stag = stagA if j % 2 == 0 else stagB
                        nc.scalar.activation(
                            stag[64 * c:64 * c + 64, w * Q * 16:(w + 1) * Q * 16],
                            ps3_tiles[j][64 * c:64 * c + 64, :],
                            AF.Relu, bias=betas_t[64 * c:64 * c + 64, 2:3], scale=1.0,
                        )

            nc.vector.tensor_scalar_min(stagA[:, :], stagA[:, :], 1.0)
            nc.vector.tensor_scalar_min(stagB[:, :], stagB[:, :], 1.0)

            # ================= L4 =================
            with tc.tile_pool(name="ps4", bufs=1, space="PSUM") as ps4pool:
                streams = [(stagA, 0), (stagA, 1), (stagB, 0), (stagB, 1)]
                ps4s = [ps4pool.tile([128, NQ], F32, name=f"ps4_{k}", tag=f"ps4_{k}")
                        for k in range(4)]
                for t in range(16):
                    for k, (stag, r) in enumerate(streams):
                        sv = stag.rearrange("p (n t) -> p n t", t=16)
                        lhsT = wl4_t[64 * r:64 * r + 64, 10 * t:10 * t + 10]
                        rhs = sv[64 * r:64 * r + 64, :, t]
                        nc.tensor.matmul(
                            ps4s[k][0:10, :], lhsT, rhs,
                            start=(t == 0), stop=(t == 15),
                            skip_group_check=True,
                            tile_position=(64 * r, 0),
                        )
                for k in range(4):
                    nc.scalar.activation(
                        out_sb[0:10, k * NQ:(k + 1) * NQ], ps4s[k][0:10, :],
                        AF.Identity, bias=betas_t[0:10, 3:4], scale=1.0,
                    )
                nc.sync.dma_start(y.ap(), out_sb[0:10, :])

    return nc


_NC_CACHE = None


def get_program():
    global _NC_CACHE
    if _NC_CACHE is None:
        nc = build_program()
        if not nc.is_finalized():
            nc.finalize()
        _NC_CACHE = nc
    return _NC_CACHE


def make_in_maps(inputs, n_cores=N_CORES):
    wdict = host_prep_weights(inputs)
    in_maps = []
    for c in range(n_cores):
        x_core = np.asarray(inputs['x_in'][c * S:(c + 1) * S], np.float32)
        m = {'x_l1': host_prep_x(x_core)}
        m.update(wdict)
        in_maps.append(m)
    return in_maps


def assemble_output(results, n_cores=N_CORES):
    """results: list of per-core dicts with y [10, 4*NQ] -> [n_cores*S, 10]."""
    out = np.zeros((n_cores * S, 10), np.float32)
    lanes = [0, 2, 1, 3]
    for c in range(n_cores):
        yc = np.asarray(results[c]['y'])  # [10, 4*NQ]
        for k, lane in enumerate(lanes):
            blk = yc[:, k * NQ:(k + 1) * NQ]  # [10, NQ]
            s_core = 4 * np.arange(NQ) + lane
            out[c * S + s_core, :] = blk.T
    return out


def kernel(**inputs) -> np.ndarray:
    from concourse.bass_utils import run_bass_kernel_spmd
    nc = get_program()
    in_maps = make_in_maps(inputs)
    res = run_bass_kernel_spmd(nc, in_maps, list(range(N_CORES)))
    return assemble_output(res.results)
